# revision 41
# baseline (speedup 1.0000x reference)
"""MedianPool2d 3x3 stride-1 reflect-pad kernel for 8 TRN2 NeuronCores.

Input:  x [16, 3, 512, 512] fp32 (full). Output: same shape, lower median
of each 3x3 window after reflect pad. Computed in fp16 (tolerance 2e-2;
fp16 quantization contributes ~2e-4 norm-relative error).

Strategy:
 - Pure data parallel: 48 images (B*C) -> 6 images per core, no collectives.
 - fp16 + pair-interleaved layout: two images per plane with their columns
   interleaved (I[:, 2c] = A[:, c], I[:, 2c+1] = B[:, c]). A +-1 column
   window shift is then a +-2 fp16 element offset = 4-byte aligned, so
   every tensor_tensor min/max qualifies for the DVE 2x_1P perf mode
   (16-bit dtype, step +-1, 4B-aligned -> 2 elem/cycle/lane). Vertical
   shifts are whole-slot offsets (1028 elems), also aligned.
 - Host staging: reflect pad to [514, 514], interleave pairs to [514, 1028];
   partition p holds rows [4p, 4p+6) of every plane (3 blocks of 6 slots),
   so all 9 window taps are free-dim offsets of one flat SBUF buffer.
 - Median-of-9 via med3(max3(col mins), med3(col meds), min3(col maxes)):
   18 min/max tensor_tensor ops per element, all on DVE (this toolchain's
   codegen rejects min/max TensorTensor on GpSimd; ACT has no two-tensor
   op), fp16 2x mode -> ~0.52 ns/elem.
 - The For_i timing loop barriers each iteration (no cross-iteration DMA
   prefetch), so the body is unrolled x2 with ping-pong input buffers:
   each half's input DMAs are issued while the other half computes, and
   only the first input chunk + last output store of a half are exposed.
   Medians are written back into the (dead) input buffer and stored from
   there, on ACT's DMA queue so SP keeps issuing input DMAs unblocked.
 - Output stays interleaved fp16 in DRAM; host de-interleaves + upcasts.
"""

import sys

for _p in ("/opt/trn_rl_repo", "/root/.axon_site/_ro/trn_rl_repo"):
    if _p not in sys.path:
        sys.path.append(_p)

import numpy as np

import concourse.bass as bass
import concourse.bacc as bacc
import concourse.mybir as mybir
from concourse.tile import TileContext

F16 = mybir.dt.float16
MIN = mybir.AluOpType.min
MAX = mybir.AluOpType.max

W = 512
WP2 = 1028           # interleaved padded pair-row width (2 * 514)
RPP = 4              # pair-rows per partition per plane
NSLOT = RPP + 2      # + top/bottom halo rows
FLAT2 = NSLOT * WP2  # 6168 fp16 per partition per plane block
CLEN2 = RPP * WP2    # 4112 flat stat/output length per block
NPAIR = 3            # image pairs (planes) per core
HALF0 = 4 * WP2      # first input-DMA chunk: slots 0-3 of block 0


def _build_bass(loop_k=1):
    nc = bacc.Bacc("TRN2", target_bir_lowering=False)
    x_d = nc.declare_dram_parameter("x", [128, NPAIR, FLAT2], F16, isOutput=False)
    o_d = nc.declare_dram_parameter("out", [128, NPAIR, CLEN2], F16, isOutput=True)

    UNROLL = 2
    assert loop_k == 1 or loop_k % UNROLL == 0, "loop_k must be 1 or even"

    import contextlib
    with TileContext(nc) as tc:
        loop_cm = (
            tc.For_i(0, loop_k // UNROLL, 1) if loop_k > 1
            else contextlib.nullcontext()
        )
        with loop_cm, tc.tile_pool(name="pool", bufs=1) as pool:
            xins = [pool.tile([128, NPAIR, FLAT2], F16, tag=f"xin{i}",
                              name=f"xin{i}")
                    for i in range(UNROLL if loop_k > 1 else 1)]
            # 3 stat lanes stored FLAT (lane b at [b*CLEN2, (b+1)*CLEN2))
            # so the all-blocks row stage runs as single contiguous 1D runs
            # (the 4-elem lane seams compute garbage that is never stored)
            L3 = NPAIR * CLEN2
            P1 = pool.tile([128, L3], F16, tag="p1")
            P2 = pool.tile([128, L3], F16, tag="p2")
            S1 = pool.tile([128, L3], F16, tag="s1")
            S2 = pool.tile([128, L3], F16, tag="s2")
            S3 = pool.tile([128, L3], F16, tag="s3")

            TT = nc.vector.tensor_tensor

            def col_stage(xin):
                """vertical min/med/max for ALL blocks: 2D input views (one
                run per block), flat 1D outputs spanning the 3 lanes"""
                h = slice(0, L3)
                v0 = xin[:, :, 0:CLEN2]
                v1 = xin[:, :, WP2 : WP2 + CLEN2]
                v2 = xin[:, :, 2 * WP2 : 2 * WP2 + CLEN2]
                TT(P1[:, h], v0, v1, MIN)
                TT(P2[:, h], v0, v1, MAX)
                TT(S1[:, h], P1[:, h], v2, MIN)      # cmin
                TT(S2[:, h], P2[:, h], v2, MAX)      # cmax
                TT(P2[:, h], P2[:, h], v2, MIN)      # t5
                TT(S3[:, h], P1[:, h], P2[:, h], MAX)  # cmed

            def row_stage():
                """merge over flat stat span [2, L3-2) - a single 1D run
                per op (lane-seam positions compute garbage, never stored);
                leaves mn2 in S3 and t3 in P1 - median = max(S3, P1)"""
                c = slice(2, L3 - 2)
                l = slice(0, L3 - 4)
                r = slice(4, L3)
                TT(P1[:, c], S1[:, l], S1[:, r], MAX)
                TT(P1[:, c], P1[:, c], S1[:, c], MAX)   # A = max3(cmin)
                TT(P2[:, c], S2[:, l], S2[:, r], MIN)
                TT(P2[:, c], P2[:, c], S2[:, c], MIN)   # C = min3(cmax)
                TT(S1[:, c], S3[:, l], S3[:, c], MIN)
                TT(S2[:, c], S3[:, l], S3[:, c], MAX)
                TT(S2[:, c], S2[:, c], S3[:, r], MIN)
                TT(S1[:, c], S1[:, c], S2[:, c], MAX)   # B = med3(cmed)
                TT(S3[:, c], P1[:, c], S1[:, c], MIN)   # mn2
                TT(P1[:, c], P1[:, c], S1[:, c], MAX)   # mx2
                TT(P1[:, c], P1[:, c], P2[:, c], MIN)   # t3

            def median():
                """median for all blocks: ONE flat op into S2 (free after
                the B stage) - 18 ops/iteration total, the algorithmic
                minimum. Seam positions get garbage, never stored."""
                c = slice(2, L3 - 2)
                TT(S2[:, c], S3[:, c], P1[:, c], MAX)

            def store():
                """one 3-run store of the medians from S2, on ACT's queue
                (must not block SP, which issues the input loads)"""
                s2v = S2.rearrange("p (l c) -> p l c", c=CLEN2)
                nc.scalar.dma_start(out=o_d[:, :, 2 : CLEN2 - 2],
                                    in_=s2v[:, :, 2 : CLEN2 - 2])

            def load(xin):
                nc.sync.dma_start(out=xin[:], in_=x_d[:])

            def emit_half(xin):
                col_stage(xin)
                row_stage()
                median()

            if len(xins) == 1:
                load(xins[0])
                emit_half(xins[0])
                store()
            else:
                # deferred store of the LAST half's medians (still in S2
                # from the previous For_i iteration; the next write of S2
                # is half 0's cmax op, which waits this store - cleared
                # ~20us before needed). First iteration stores junk that
                # later iterations overwrite; the correctness path has no
                # loop. The iteration barrier never waits on a store.
                store()
                # xin1's load transfers while half 0 computes
                load(xins[1])
                emit_half(xins[0])
                store()
                emit_half(xins[1])
                # software-pipelined load of half 0's input for the NEXT
                # iteration: waits only this iteration's xin0 column reads,
                # so the transfer fully overlaps this iteration's compute
                load(xins[0])
    return nc


_NC_CACHE = None


def _get_nc():
    global _NC_CACHE
    if _NC_CACHE is None:
        nc = _build_bass()
        nc.compile()
        _NC_CACHE = nc
    return _NC_CACHE


def _stage_core(imgs):
    """imgs: [6, 512, 512] float -> staged [128, NPAIR, FLAT2] fp16: pairs
    reflect-padded, column-interleaved, 6-row sliding slots per partition."""
    imgs = np.asarray(imgs, dtype=np.float16)
    xp = np.pad(imgs, ((0, 0), (1, 1), (1, 1)), mode="reflect")  # [6, 514, 514]
    inter = np.empty((NPAIR, 514, WP2), dtype=np.float16)
    inter[:, :, 0::2] = xp[0::2]
    inter[:, :, 1::2] = xp[1::2]
    idx = np.arange(128)[:, None] * RPP + np.arange(NSLOT)[None, :]  # [128, 6]
    blocks = inter[:, idx, :]  # [NPAIR, 128, 6, 1028]
    staged = blocks.reshape(NPAIR, 128, FLAT2).transpose(1, 0, 2)
    return np.ascontiguousarray(staged)


def _unstage_core(out_d):
    """out_d: [128, NPAIR, CLEN2] fp16 -> [6, 512, 512] fp32."""
    o = out_d.transpose(1, 0, 2).reshape(NPAIR, 128, RPP, WP2)[:, :, :, 2 : 2 + 2 * W]
    o = o.reshape(NPAIR, 512, 2 * W)
    res = np.empty((6, 512, 512), dtype=np.float32)
    res[0::2] = o[:, :, 0::2].astype(np.float32)
    res[1::2] = o[:, :, 1::2].astype(np.float32)
    return res


def run(x, trace=False):
    """x: [16,3,512,512] fp32 -> (out [16,3,512,512] fp32, exec_time_ns|None)"""
    from concourse.bass_utils import run_bass_kernel_spmd

    x = np.ascontiguousarray(np.asarray(x, dtype=np.float32))
    B, C, H, Wd = x.shape
    imgs = x.reshape(8, 6, H, Wd)
    in_maps = [{"x": _stage_core(imgs[i])} for i in range(8)]
    nc = _get_nc()
    res = run_bass_kernel_spmd(nc, in_maps, list(range(8)), trace=trace)
    out = np.stack([_unstage_core(res.results[i]["out"]) for i in range(8)])
    return out.reshape(B, C, H, Wd), res.exec_time_ns


def kernel(x):
    out, _ = run(x, trace=False)
    return out


# revision 42
# speedup vs baseline: 1.0743x; 1.0743x over previous
"""MedianPool2d 3x3 stride-1 reflect-pad kernel for 8 TRN2 NeuronCores.

Input:  x [16, 3, 512, 512] fp32 (full). Output: same shape, lower median
of each 3x3 window after reflect pad. Computed in fp16 (tolerance 2e-2;
fp16 quantization contributes ~2e-4 norm-relative error).

Strategy:
 - Pure data parallel: 48 images (B*C) -> 6 images per core, no collectives.
 - fp16 + pair-interleaved layout: two images per plane with their columns
   interleaved (I[:, 2c] = A[:, c], I[:, 2c+1] = B[:, c]). A +-1 column
   window shift is then a +-2 fp16 element offset = 4-byte aligned, so
   every tensor_tensor min/max qualifies for the DVE 2x_1P perf mode
   (16-bit dtype, step +-1, 4B-aligned -> 2 elem/cycle/lane). Vertical
   shifts are whole-slot offsets (1028 elems), also aligned.
 - Host staging: reflect pad to [514, 514], interleave pairs to [514, 1028];
   partition p holds rows [4p, 4p+6) of every plane (3 blocks of 6 slots),
   so all 9 window taps are free-dim offsets of one flat SBUF buffer.
 - Median-of-9 via med3(max3(col mins), med3(col meds), min3(col maxes)):
   18 min/max tensor_tensor ops per element, all on DVE (this toolchain's
   codegen rejects min/max TensorTensor on GpSimd; ACT has no two-tensor
   op), fp16 2x mode -> ~0.52 ns/elem.
 - The For_i timing loop barriers each iteration (no cross-iteration DMA
   prefetch via the scheduler), so DMAs are software-pipelined in program
   order instead: the body is unrolled x2 with ping-pong input buffers,
   each half's load is emitted while the other half computes (half 0's at
   the body TAIL for the next iteration), and the last half's store is
   deferred to the body top of the next iteration - the barrier never
   waits on a DMA. Medians land in S2 (free after the merge) and are
   stored from ACT's DMA queue so SP keeps issuing input loads unblocked.
 - Output stays interleaved fp16 in DRAM; host de-interleaves + upcasts.
"""

import sys

for _p in ("/opt/trn_rl_repo", "/root/.axon_site/_ro/trn_rl_repo"):
    if _p not in sys.path:
        sys.path.append(_p)

import numpy as np

import concourse.bass as bass
import concourse.bacc as bacc
import concourse.mybir as mybir
from concourse.tile import TileContext

F16 = mybir.dt.float16
MIN = mybir.AluOpType.min
MAX = mybir.AluOpType.max

W = 512
WP2 = 1028           # interleaved padded pair-row width (2 * 514)
RPP = 4              # pair-rows per partition per plane
NSLOT = RPP + 2      # + top/bottom halo rows
FLAT2 = NSLOT * WP2  # 6168 fp16 per partition per plane block
CLEN2 = RPP * WP2    # 4112 flat stat/output length per block
NPAIR = 3            # image pairs (planes) per core
HALF0 = 4 * WP2      # first input-DMA chunk: slots 0-3 of block 0


def _build_bass(loop_k=1):
    nc = bacc.Bacc("TRN2", target_bir_lowering=False)
    x_d = nc.declare_dram_parameter("x", [128, NPAIR, FLAT2], F16, isOutput=False)
    o_d = nc.declare_dram_parameter("out", [128, NPAIR, CLEN2], F16, isOutput=True)

    UNROLL = 2
    assert loop_k == 1 or loop_k % UNROLL == 0, "loop_k must be 1 or even"

    import contextlib
    with TileContext(nc) as tc:
        loop_cm = (
            tc.For_i(0, loop_k // UNROLL, 1) if loop_k > 1
            else contextlib.nullcontext()
        )
        with loop_cm, tc.tile_pool(name="pool", bufs=1) as pool:
            xins = [pool.tile([128, NPAIR, FLAT2], F16, tag=f"xin{i}",
                              name=f"xin{i}")
                    for i in range(UNROLL if loop_k > 1 else 1)]
            # 3 stat lanes stored FLAT (lane b at [b*CLEN2, (b+1)*CLEN2))
            # so the all-blocks row stage runs as single contiguous 1D runs
            # (the 4-elem lane seams compute garbage that is never stored)
            L3 = NPAIR * CLEN2
            P1 = pool.tile([128, L3], F16, tag="p1")
            P2 = pool.tile([128, L3], F16, tag="p2")
            S1 = pool.tile([128, L3], F16, tag="s1")
            S2 = pool.tile([128, L3], F16, tag="s2")
            S3 = pool.tile([128, L3], F16, tag="s3")

            TT = nc.vector.tensor_tensor

            def col_stage(xin):
                """vertical min/med/max for ALL blocks: 2D input views (one
                run per block), flat 1D outputs spanning the 3 lanes"""
                h = slice(0, L3)
                v0 = xin[:, :, 0:CLEN2]
                v1 = xin[:, :, WP2 : WP2 + CLEN2]
                v2 = xin[:, :, 2 * WP2 : 2 * WP2 + CLEN2]
                TT(P1[:, h], v0, v1, MIN)
                TT(P2[:, h], v0, v1, MAX)
                TT(S1[:, h], P1[:, h], v2, MIN)      # cmin
                TT(S2[:, h], P2[:, h], v2, MAX)      # cmax
                TT(P2[:, h], P2[:, h], v2, MIN)      # t5
                TT(S3[:, h], P1[:, h], P2[:, h], MAX)  # cmed

            def row_stage():
                """merge over flat stat span [2, L3-2) - a single 1D run
                per op (lane-seam positions compute garbage, never stored);
                leaves mn2 in S3 and t3 in P1 - median = max(S3, P1)"""
                c = slice(2, L3 - 2)
                l = slice(0, L3 - 4)
                r = slice(4, L3)
                TT(P1[:, c], S1[:, l], S1[:, r], MAX)
                TT(P1[:, c], P1[:, c], S1[:, c], MAX)   # A = max3(cmin)
                TT(P2[:, c], S2[:, l], S2[:, r], MIN)
                TT(P2[:, c], P2[:, c], S2[:, c], MIN)   # C = min3(cmax)
                TT(S1[:, c], S3[:, l], S3[:, c], MIN)
                TT(S2[:, c], S3[:, l], S3[:, c], MAX)
                TT(S2[:, c], S2[:, c], S3[:, r], MIN)
                TT(S1[:, c], S1[:, c], S2[:, c], MAX)   # B = med3(cmed)
                TT(S3[:, c], P1[:, c], S1[:, c], MIN)   # mn2
                TT(P1[:, c], P1[:, c], S1[:, c], MAX)   # mx2
                TT(P1[:, c], P1[:, c], P2[:, c], MIN)   # t3

            def median():
                """median for all blocks: ONE flat op into S2 (free after
                the B stage) - 18 ops/iteration total, the algorithmic
                minimum. Seam positions get garbage, never stored."""
                c = slice(2, L3 - 2)
                TT(S2[:, c], S3[:, c], P1[:, c], MAX)

            def store():
                """one 3-run store of the medians from S2, on ACT's queue
                (must not block SP, which issues the input loads)"""
                s2v = S2.rearrange("p (l c) -> p l c", c=CLEN2)
                nc.scalar.dma_start(out=o_d[:, :, 2 : CLEN2 - 2],
                                    in_=s2v[:, :, 2 : CLEN2 - 2])

            def load(xin):
                nc.sync.dma_start(out=xin[:], in_=x_d[:])

            def emit_half(xin):
                col_stage(xin)
                row_stage()
                median()

            if len(xins) == 1:
                load(xins[0])
                emit_half(xins[0])
                store()
            else:
                # deferred store of the LAST half's medians (still in S2
                # from the previous For_i iteration; the next write of S2
                # is half 0's cmax op, which waits this store - cleared
                # ~20us before needed). First iteration stores junk that
                # later iterations overwrite; the correctness path has no
                # loop. The iteration barrier never waits on a store.
                store()
                # xin1's load transfers while half 0 computes
                load(xins[1])
                emit_half(xins[0])
                store()
                emit_half(xins[1])
                # software-pipelined load of half 0's input for the NEXT
                # iteration: waits only this iteration's xin0 column reads,
                # so the transfer fully overlaps this iteration's compute
                load(xins[0])
    return nc


_NC_CACHE = None


def _get_nc():
    global _NC_CACHE
    if _NC_CACHE is None:
        nc = _build_bass()
        nc.compile()
        _NC_CACHE = nc
    return _NC_CACHE


def _stage_core(imgs):
    """imgs: [6, 512, 512] float -> staged [128, NPAIR, FLAT2] fp16: pairs
    reflect-padded, column-interleaved, 6-row sliding slots per partition."""
    imgs = np.asarray(imgs, dtype=np.float16)
    xp = np.pad(imgs, ((0, 0), (1, 1), (1, 1)), mode="reflect")  # [6, 514, 514]
    inter = np.empty((NPAIR, 514, WP2), dtype=np.float16)
    inter[:, :, 0::2] = xp[0::2]
    inter[:, :, 1::2] = xp[1::2]
    idx = np.arange(128)[:, None] * RPP + np.arange(NSLOT)[None, :]  # [128, 6]
    blocks = inter[:, idx, :]  # [NPAIR, 128, 6, 1028]
    staged = blocks.reshape(NPAIR, 128, FLAT2).transpose(1, 0, 2)
    return np.ascontiguousarray(staged)


def _unstage_core(out_d):
    """out_d: [128, NPAIR, CLEN2] fp16 -> [6, 512, 512] fp32."""
    o = out_d.transpose(1, 0, 2).reshape(NPAIR, 128, RPP, WP2)[:, :, :, 2 : 2 + 2 * W]
    o = o.reshape(NPAIR, 512, 2 * W)
    res = np.empty((6, 512, 512), dtype=np.float32)
    res[0::2] = o[:, :, 0::2].astype(np.float32)
    res[1::2] = o[:, :, 1::2].astype(np.float32)
    return res


def run(x, trace=False):
    """x: [16,3,512,512] fp32 -> (out [16,3,512,512] fp32, exec_time_ns|None)"""
    from concourse.bass_utils import run_bass_kernel_spmd

    x = np.ascontiguousarray(np.asarray(x, dtype=np.float32))
    B, C, H, Wd = x.shape
    imgs = x.reshape(8, 6, H, Wd)
    in_maps = [{"x": _stage_core(imgs[i])} for i in range(8)]
    nc = _get_nc()
    res = run_bass_kernel_spmd(nc, in_maps, list(range(8)), trace=trace)
    out = np.stack([_unstage_core(res.results[i]["out"]) for i in range(8)])
    return out.reshape(B, C, H, Wd), res.exec_time_ns


def kernel(x):
    out, _ = run(x, trace=False)
    return out
